# revision 1
# baseline (speedup 1.0000x reference)
"""Trainium2 Bass kernel for nn_ConvJac: 32 Jacobi sweeps of a
variable-coefficient 5-point stencil over a batch of 16 images of 512x512.

Strategy
--------
Data-parallel over the batch: 16 images over 8 NeuronCores -> 2 images per
core, no inter-core communication (the stencil never crosses images).

Per-core layout: the 2 images are stacked into a 1024x512 row block; SBUF
partition p holds 8 consecutive rows flattened along the free dim
(8*512 = 4096 f16 values), with 512-element halo columns on both sides
holding the neighbor partitions' boundary rows.  In this layout all four
stencil neighbors are free-dim offset reads (+-1, +-512); the only
cross-partition traffic is a 1-row halo exchange per sweep, done with
SBUF->SBUF DMAs (zero coefficients at image edges absorb every wrap
artifact, so no masking is needed).

Per sweep (split into four 2-bank chunks whose processing order rotates by
+1 every sweep, so each sweep's first chunk depends only on the previous
sweep's EARLY chunks -> no inter-sweep pipeline bubble):
  VectorE   2 fused f16 products per chunk (2x DVE mode): one two-window
            instruction computes both horizontal products q1/q2 (the +-1
            shifts are pre-folded into the coefficient slots so every
            operand stays 4-byte aligned), a second computes both vertical
            products t3/t4.
  TensorE   5 identity matmuls per PSUM bank accumulate the 4 products +
            b' in fp32 (exact adds); redundant identity weight reloads are
            deduplicated post-build.
  ScalarE   evacuates PSUM -> u_next (f16) per bank
  DMA       3 halo row copies
Coefficients (harmonic-mean face conductivities / diag) are computed on
device from K once at setup, in fp32.
"""

import numpy as np

import concourse.bacc as bacc
import concourse.bass as bass
import concourse.mybir as mybir
from concourse.tile import TileContext
from concourse.bass_utils import run_bass_kernel_spmd

P = 128          # SBUF partitions
W = 512          # image width
RPP = 8          # rows per partition (1024 rows / 128 partitions)
F = RPP * W      # interior free-dim size (4096)
H0 = W           # halo width (one row)
UW = F + 2 * H0  # u tile width with halos
SW = F + 2       # q-slot array width (4098)
N_CORES = 8

_prog_cache = {}


def _dedup_ldweights(nc):
    """Drop back-to-back InstLdweights that reload the identical stationary
    tensor (every matmul here uses the same 128x128 identity).  Only
    sync-free duplicates are removed, so the semaphore graph is unchanged;
    the PE keeps the previously loaded weights."""
    for f in nc.m.functions:
        for bb in f.blocks:
            key = None
            keep = []
            for inst in bb.instructions:
                nm = type(inst).__name__
                if nm == "InstLdweights":
                    a = inst.ins[0]
                    k = (a.memref, a.offset, str(a.ap))
                    si = inst.sync_info
                    clean = not (si and (list(si.on_wait or [])
                                         or list(si.on_update or [])))
                    if k == key and clean:
                        continue
                    key = k
                keep.append(inst)
            if len(keep) != len(bb.instructions):
                bb.instructions = keep


def _win2(tile_ap, off, step, n):
    """AP of shape [128, 2, n]: two windows of `n` contiguous elements at
    element offsets off and off+step within the tile."""
    base = tile_ap.copy()
    ap0 = list(base.ap[0])
    return bass.AP(tensor=base.tensor, offset=base.offset + off,
                   ap=[ap0, [step, 2], [1, n]])


def _build(iters: int):
    nc = bacc.Bacc("TRN2", target_bir_lowering=False, name=f"convjac{iters}")
    f32, f16 = mybir.dt.float32, mybir.dt.float16
    mult = mybir.AluOpType.mult

    u_in = nc.dram_tensor("u_in", [P, F], f32, kind="ExternalInput")
    b_in = nc.dram_tensor("b_in", [P, F], f32, kind="ExternalInput")
    k_in = nc.dram_tensor("k_in", [P, F], f32, kind="ExternalInput")
    ident = nc.dram_tensor("ident", [P, P], f16, kind="ExternalInput")
    out = nc.dram_tensor("out", [P, F], f32, kind="ExternalOutput")

    with TileContext(nc) as tc:
        with tc.tile_pool(name="pers", bufs=1) as pers:
            u0 = pers.tile([P, UW], f16, tag="u0")
            u1 = pers.tile([P, UW], f16, tag="u1")
            # LRh = [Lh | Rh] slot arrays: Lh[s] = cL[s-1], Rh[s] = cR[s-1];
            # the slot shift keeps every DVE u-read 4-byte aligned.
            LRh = pers.tile([P, 2 * SW], f16, tag="LRh")
            # CDU = [cD | cU]
            CDU = pers.tile([P, 2 * F], f16, tag="CDU")
            cB = pers.tile([P, F], f16, tag="cB")
            idt = pers.tile([P, P], f16, tag="idt")
            nc.sync.dma_start(out=idt[:], in_=ident[:])

            with tc.tile_pool(name="setup", bufs=1) as sp:
                kst = sp.tile([P, F + H0], f32, tag="kst")
                rt = sp.tile([P, F + 2], f32, tag="rt")
                ut = sp.tile([P, F + H0], f32, tag="ut")
                s1 = sp.tile([P, F], f32, tag="s1")
                s2 = sp.tile([P, F], f32, tag="s2")

                # K with a one-row halo; 1e30 at image bottoms so
                # 1/(lbd + ~0) realizes the Dirichlet face 2K.
                nc.gpsimd.memset(kst[:, F:F + H0], 1e30)
                nc.sync.dma_start(out=kst[:, 0:F], in_=k_in[:])
                nc.sync.dma_start(out=kst[0:63, F:F + H0], in_=k_in[1:64, 0:H0])
                nc.sync.dma_start(out=kst[64:127, F:F + H0], in_=k_in[65:128, 0:H0])
                # lbd = 1/K (in place)
                nc.vector.reciprocal_approx_fast(out=kst[:], in_=kst[:])
                # horizontal half-faces: rt[s] = 1/(lbd[s-1]+lbd[s])
                nc.vector.tensor_add(s1[:], kst[:, 0:F], kst[:, 1:F + 1])
                nc.vector.reciprocal_approx_fast(out=rt[:, 1:F + 1], in_=s1[:])
                nc.vector.memset(rt[:, 0:F + 1:W], 0.0)  # cross-row faces
                nc.vector.memset(rt[:, F + 1:F + 2], 0.0)
                # vertical half-faces: ut[512+x] = 1/(lbd[x]+lbd[x+512])
                nc.vector.tensor_add(s2[:], kst[:, 0:F], kst[:, H0:F + H0])
                nc.vector.reciprocal_approx_fast(out=ut[:, H0:F + H0], in_=s2[:])
                nc.gpsimd.memset(ut[:, 0:H0], 0.0)
                nc.sync.dma_start(out=ut[1:64, 0:H0], in_=ut[0:63, F:F + H0])
                nc.sync.dma_start(out=ut[65:128, 0:H0], in_=ut[64:127, F:F + H0])
                # rd = 1/(half-diagonal)  (into s1, in place)
                nc.vector.tensor_add(s1[:], rt[:, 0:F], rt[:, 1:F + 1])
                nc.vector.tensor_add(s2[:], ut[:, 0:F], ut[:, H0:F + H0])
                nc.vector.tensor_add(s1[:], s1[:], s2[:])
                nc.vector.reciprocal_approx_fast(out=s1[:], in_=s1[:])
                # normalized coefficients (f32 -> f16)
                nc.vector.tensor_mul(LRh[:, 1:F + 1], rt[:, 0:F], s1[:])
                nc.vector.tensor_mul(LRh[:, SW + 1:SW + F + 1], rt[:, 1:F + 1], s1[:])
                nc.vector.tensor_mul(CDU[:, 0:F], ut[:, 0:F], s1[:])
                nc.vector.tensor_mul(CDU[:, F:2 * F], ut[:, H0:F + H0], s1[:])
                for col in (0, F + 1, SW, SW + F + 1):
                    nc.gpsimd.memset(LRh[:, col:col + 1], 0.0)
                # b' = b/(2*half-diag)
                nc.sync.dma_start(out=s2[:], in_=b_in[:])
                nc.vector.scalar_tensor_tensor(
                    out=cB[:], in0=s2[:], scalar=0.5, in1=s1[:], op0=mult, op1=mult
                )
                # initial u (f16) + halos
                nc.sync.dma_start(out=s2[:], in_=u_in[:])
                for uu in (u0, u1):
                    nc.gpsimd.memset(uu[:, 0:H0], 0.0)
                    nc.gpsimd.memset(uu[:, F + H0:UW], 0.0)
                nc.vector.tensor_copy(out=u0[:, H0:F + H0], in_=s2[:])
                nc.sync.dma_start(out=u0[1:128, 0:H0], in_=u0[0:127, F:F + H0])
                nc.sync.dma_start(out=u0[0:63, F + H0:UW], in_=u0[1:64, H0:2 * H0])
                nc.sync.dma_start(out=u0[64:127, F + H0:UW], in_=u0[65:128, H0:2 * H0])

            CW = F // 4  # chunk width (1024), 2 PSUM banks
            with tc.tile_pool(name="work", bufs=2) as wp, \
                 tc.tile_pool(name="psum", bufs=1, space="PSUM") as pp:
                bufs = [u0, u1]
                ps = pp.tile([P, F], f32, tag="ps")
                for it in range(iters):
                    src = bufs[it % 2]
                    dst = bufs[1 - it % 2]
                    q12 = wp.tile([P, 2 * SW], f16, tag="q12")
                    t34 = wp.tile([P, 2 * F], f16, tag="t34")
                    for j in range(4):
                        c = (it + j) % 4          # chunk index this step
                        x0 = c * CW               # interior col base
                        # q-slot write range: starts at x0 for the first
                        # chunk of the sweep (covers its own lead slots) and
                        # for chunk 0 (its predecessor chunk 3 wraps and
                        # never covers slots 0..2); x0+2 otherwise (the
                        # previous chunk of the SAME sweep covered them).
                        s0 = x0 if (j == 0 or c == 0) else x0 + 2
                        n = x0 + CW + 2 - s0
                        # one op, two windows: q1[s]=Lh[s]*u[s-2] and
                        # q2[s]=Rh[s]*u[s] for s in [s0, s0+n)
                        nc.vector.tensor_mul(
                            _win2(q12[:], s0, SW, n),
                            _win2(LRh[:], s0, SW, n),
                            _win2(src[:], H0 - 2 + s0, 2, n))
                        # one op, two windows: t3=cD*u_d, t4=cU*u_u
                        nc.vector.tensor_mul(
                            _win2(t34[:], x0, F, CW),
                            _win2(CDU[:], x0, F, CW),
                            _win2(src[:], x0, 2 * H0, CW))
                        for k in (2 * c, 2 * c + 1):
                            a, e = k * W, k * W + W
                            mm = nc.tensor.matmul
                            mm(ps[:, a:e], lhsT=idt[:], rhs=cB[:, a:e], start=True, stop=False)
                            mm(ps[:, a:e], lhsT=idt[:], rhs=q12[:, a + 1:e + 1], start=False, stop=False)
                            mm(ps[:, a:e], lhsT=idt[:], rhs=q12[:, SW + a + 1:SW + e + 1], start=False, stop=False)
                            mm(ps[:, a:e], lhsT=idt[:], rhs=t34[:, a:e], start=False, stop=False)
                            mm(ps[:, a:e], lhsT=idt[:], rhs=t34[:, F + a:F + e], start=False, stop=True)
                            nc.scalar.copy(out=dst[:, H0 + a:H0 + e], in_=ps[:, a:e])
                        if c == 0:
                            # back halos need dst row 0 (bank 0, just written)
                            nc.sync.dma_start(out=dst[0:63, F + H0:UW],
                                              in_=dst[1:64, H0:2 * H0])
                            nc.sync.dma_start(out=dst[64:127, F + H0:UW],
                                              in_=dst[65:128, H0:2 * H0])
                        if c == 3:
                            # front halo needs dst row 7 (bank 7, just written)
                            nc.sync.dma_start(out=dst[1:128, 0:H0],
                                              in_=dst[0:127, F:F + H0])

                res = bufs[iters % 2]
                ost = wp.tile([P, F], f32, tag="ost", bufs=1)
                nc.vector.tensor_copy(out=ost[:], in_=res[:, H0:F + H0])
                nc.sync.dma_start(out=out[:], in_=ost[:])

    _dedup_ldweights(nc)
    nc.compile()
    return nc


def _get_program(iters: int):
    if iters not in _prog_cache:
        _prog_cache[iters] = _build(iters)
    return _prog_cache[iters]


def _make_in_maps(u, b, K):
    u = np.ascontiguousarray(u, dtype=np.float32)
    b = np.ascontiguousarray(b, dtype=np.float32)
    K = np.ascontiguousarray(K, dtype=np.float32)
    ident = np.eye(P, dtype=np.float16)
    in_maps = []
    for c in range(N_CORES):
        sl = slice(2 * c, 2 * c + 2)
        in_maps.append({
            "u_in": u[sl].reshape(P, F),
            "b_in": b[sl].reshape(P, F),
            "k_in": K[sl].reshape(P, F),
            "ident": ident,
        })
    return in_maps


def kernel(max_iter, u, b, K):
    iters = int(max_iter)
    nc = _get_program(iters)
    in_maps = _make_in_maps(u, b, K)
    res = run_bass_kernel_spmd(nc, in_maps, core_ids=list(range(N_CORES)))
    out = np.concatenate(
        [r["out"].reshape(2, W, W) for r in res.results], axis=0
    ).astype(np.float32)
    return out



# revision 26
# speedup vs baseline: 163.9416x; 163.9416x over previous
"""Trainium2 Bass kernel for nn_ConvJac: 32 Jacobi sweeps of a
variable-coefficient 5-point stencil over a batch of 16 images of 512x512.

Strategy
--------
Data-parallel over the batch: 16 images over 8 NeuronCores -> 2 images per
core, no inter-core communication (the stencil never crosses images).

Per-core layout: the 2 images are stacked into a 1024x512 row block; SBUF
partition p holds 8 consecutive rows flattened along the free dim
(8*512 = 4096 f16 values), with 512-element halo columns on both sides
holding the neighbor partitions' boundary rows.  In this layout all four
stencil neighbors are free-dim offset reads (+-1, +-512); the only
cross-partition traffic is a 1-row halo exchange per sweep, done with
SBUF->SBUF DMAs (zero coefficients at image edges absorb every wrap
artifact, so no masking is needed).

Per sweep the row block is processed in four 2-bank chunks whose order
rotates by +1 every sweep (chunk c of sweep s+1 depends only on sweep s's
EARLY chunks -> no inter-sweep pipeline bubble).  Engine split:
  VectorE   2 three-level-AP f16 products per chunk PAIR (2x DVE mode):
            one instruction computes the horizontal products q1/q2 for two
            chunks at once ([chunk, slot, elem] access pattern; the +-1
            shifts are pre-folded into the coefficient slots so every
            operand stays 4-byte aligned), a second computes the vertical
            products t3/t4 the same way.
  TensorE   5 identity matmuls per PSUM bank accumulate the 4 products +
            b' in fp32 (exact adds); redundant identity weight reloads
            are deduplicated post-build.
  ScalarE   evacuates PSUM -> u_next (f16) per chunk (1024 wide)
  DMA       3 halo row copies per sweep
Coefficients (harmonic-mean face conductivities / diag) are computed on
device from K once at setup, in fp32.
"""

import numpy as np

import concourse.bacc as bacc
import concourse.bass as bass
import concourse.mybir as mybir
from concourse.tile import TileContext
from concourse.bass_utils import run_bass_kernel_spmd

P = 128          # SBUF partitions
W = 512          # image width
RPP = 8          # rows per partition (1024 rows / 128 partitions)
F = RPP * W      # interior free-dim size (4096)
H0 = W           # halo width (one row)
UW = F + 2 * H0  # u tile width with halos
SW = F + 2       # q-slot array width (4098)
CW = F // 4      # chunk width (1024), 2 PSUM banks
N_CORES = 8

_prog_cache = {}


def _dedup_ldweights(nc):
    """Drop back-to-back InstLdweights that reload the identical stationary
    tensor (every matmul here uses the same 128x128 identity).  Only
    sync-free duplicates are removed, so the semaphore graph is unchanged;
    the PE keeps the previously loaded weights."""
    for f in nc.m.functions:
        for bb in f.blocks:
            key = None
            keep = []
            for inst in bb.instructions:
                nm = type(inst).__name__
                if nm == "InstLdweights":
                    a = inst.ins[0]
                    k = (a.memref, a.offset, str(a.ap))
                    si = inst.sync_info
                    clean = not (si and (list(si.on_wait or [])
                                         or list(si.on_update or [])))
                    if k == key and clean:
                        continue
                    key = k
                keep.append(inst)
            if len(keep) != len(bb.instructions):
                bb.instructions = keep


def _bank_order(c):
    """Bank processing order within chunk c: the bank holding a halo source
    row (bank 7 -> front halo) is processed first so its evacuation and the
    halo DMA fire as early as possible."""
    return (7, 6) if c == 3 else (2 * c, 2 * c + 1)


def _win2(tile_ap, off, step, n):
    """AP of shape [128, 2, n]: two windows of `n` contiguous elements at
    element offsets off and off+step within the tile."""
    base = tile_ap.copy()
    ap0 = list(base.ap[0])
    return bass.AP(tensor=base.tensor, offset=base.offset + off,
                   ap=[ap0, [step, 2], [1, n]])


def _win4(tile_ap, off, cstep, wstep, n):
    """AP of shape [128, 2, 2, n]: (chunk-pair, window-pair, elems) —
    two chunks at element offsets off and off+cstep, each with two windows
    of `n` contiguous elements wstep apart."""
    base = tile_ap.copy()
    ap0 = list(base.ap[0])
    return bass.AP(tensor=base.tensor, offset=base.offset + off,
                   ap=[ap0, [cstep, 2], [wstep, 2], [1, n]])


def _build(iters: int, act_wide: bool = False, dve_grouped: bool = False,
           row_chunks: bool = False):
    nc = bacc.Bacc("TRN2", target_bir_lowering=False, name=f"convjac{iters}")
    f32, f16 = mybir.dt.float32, mybir.dt.float16
    mult = mybir.AluOpType.mult

    u_in = nc.dram_tensor("u_in", [P, F], f32, kind="ExternalInput")
    b_in = nc.dram_tensor("b_in", [P, F], f32, kind="ExternalInput")
    k_in = nc.dram_tensor("k_in", [P, F], f32, kind="ExternalInput")
    ident = nc.dram_tensor("ident", [P, P], f16, kind="ExternalInput")
    out = nc.dram_tensor("out", [P, F], f32, kind="ExternalOutput")

    with TileContext(nc) as tc:
        with tc.tile_pool(name="pers", bufs=1) as pers:
            u0 = pers.tile([P, UW], f16, tag="u0")
            u1 = pers.tile([P, UW], f16, tag="u1")
            # LRh = [Lh | Rh] slot arrays: Lh[s] = cL[s-1], Rh[s] = cR[s-1];
            # the slot shift keeps every DVE u-read 4-byte aligned.
            LRh = pers.tile([P, 2 * SW], f16, tag="LRh")
            # CDU = [cD | cU]
            CDU = pers.tile([P, 2 * F], f16, tag="CDU")
            cB = pers.tile([P, F], f16, tag="cB")
            idt = pers.tile([P, P], f16, tag="idt")
            nc.sync.dma_start(out=idt[:], in_=ident[:])

            with tc.tile_pool(name="setup", bufs=1) as sp:
                kst = sp.tile([P, F + H0], f32, tag="kst")
                rt = sp.tile([P, F + 2], f32, tag="rt")
                ut = sp.tile([P, F + H0], f32, tag="ut")
                s1 = sp.tile([P, F], f32, tag="s1")
                s2 = sp.tile([P, F], f32, tag="s2")

                # K with a one-row halo; 1e30 at image bottoms so
                # 1/(lbd + ~0) realizes the Dirichlet face 2K.
                nc.gpsimd.memset(kst[:, F:F + H0], 1e30)
                nc.sync.dma_start(out=kst[:, 0:F], in_=k_in[:])
                nc.sync.dma_start(out=kst[0:63, F:F + H0], in_=k_in[1:64, 0:H0])
                nc.sync.dma_start(out=kst[64:127, F:F + H0], in_=k_in[65:128, 0:H0])
                # lbd = 1/K (in place)
                nc.vector.reciprocal_approx_fast(out=kst[:], in_=kst[:])
                # horizontal half-faces: rt[s] = 1/(lbd[s-1]+lbd[s])
                nc.vector.tensor_add(s1[:], kst[:, 0:F], kst[:, 1:F + 1])
                nc.vector.reciprocal_approx_fast(out=rt[:, 1:F + 1], in_=s1[:])
                nc.vector.memset(rt[:, 0:F + 1:W], 0.0)  # cross-row faces
                nc.vector.memset(rt[:, F + 1:F + 2], 0.0)
                # vertical half-faces: ut[512+x] = 1/(lbd[x]+lbd[x+512])
                nc.vector.tensor_add(s2[:], kst[:, 0:F], kst[:, H0:F + H0])
                nc.vector.reciprocal_approx_fast(out=ut[:, H0:F + H0], in_=s2[:])
                nc.gpsimd.memset(ut[:, 0:H0], 0.0)
                nc.sync.dma_start(out=ut[1:64, 0:H0], in_=ut[0:63, F:F + H0])
                nc.sync.dma_start(out=ut[65:128, 0:H0], in_=ut[64:127, F:F + H0])
                # rd = 1/(half-diagonal)  (into s1, in place)
                nc.vector.tensor_add(s1[:], rt[:, 0:F], rt[:, 1:F + 1])
                nc.vector.tensor_add(s2[:], ut[:, 0:F], ut[:, H0:F + H0])
                nc.vector.tensor_add(s1[:], s1[:], s2[:])
                nc.vector.reciprocal_approx_fast(out=s1[:], in_=s1[:])
                # normalized coefficients (f32 -> f16)
                nc.vector.tensor_mul(LRh[:, 1:F + 1], rt[:, 0:F], s1[:])
                nc.vector.tensor_mul(LRh[:, SW + 1:SW + F + 1], rt[:, 1:F + 1], s1[:])
                nc.vector.tensor_mul(CDU[:, 0:F], ut[:, 0:F], s1[:])
                nc.vector.tensor_mul(CDU[:, F:2 * F], ut[:, H0:F + H0], s1[:])
                for col in (0, F + 1, SW, SW + F + 1):
                    nc.gpsimd.memset(LRh[:, col:col + 1], 0.0)
                # b' = b/(2*half-diag)
                nc.sync.dma_start(out=s2[:], in_=b_in[:])
                nc.vector.scalar_tensor_tensor(
                    out=cB[:], in0=s2[:], scalar=0.5, in1=s1[:], op0=mult, op1=mult
                )
                # initial u (f16) + halos
                nc.sync.dma_start(out=s2[:], in_=u_in[:])
                for uu in (u0, u1):
                    nc.gpsimd.memset(uu[:, 0:H0], 0.0)
                    nc.gpsimd.memset(uu[:, F + H0:UW], 0.0)
                nc.vector.tensor_copy(out=u0[:, H0:F + H0], in_=s2[:])
                nc.sync.dma_start(out=u0[1:128, 0:H0], in_=u0[0:127, F:F + H0])
                nc.sync.dma_start(out=u0[0:63, F + H0:UW], in_=u0[1:64, H0:2 * H0])
                nc.sync.dma_start(out=u0[64:127, F + H0:UW], in_=u0[65:128, H0:2 * H0])

            with tc.tile_pool(name="work", bufs=2) as wp, \
                 tc.tile_pool(name="psum", bufs=1, space="PSUM") as pp:
                bufs = [u0, u1]
                ps = pp.tile([P, F], f32, tag="ps")
                for it in range(iters):
                    src = bufs[it % 2]
                    dst = bufs[1 - it % 2]
                    q12 = wp.tile([P, 2 * SW], f16, tag="q12")
                    t34 = wp.tile([P, 2 * F], f16, tag="t34")
                    if row_chunks:
                        # row-granular processing (1 row = 1 PSUM bank per
                        # step): each row's next-sweep inputs span only rows
                        # r-1..r+1, so the cross-sweep dependency chain is a
                        # quarter of a sweep instead of three quarters, and
                        # the fine interleave keeps the PE continuously fed
                        # (holding its fast p-state).  Halo-coupled rows 0
                        # and 7 run first so both halo DMAs fire early.
                        row_order = [1, 0, 2, 7, 3, 6, 4, 5]
                        for r in row_order:
                            x0 = r * W
                            jj = row_order.index(r)
                            pred_earlier = r > 0 and row_order.index(r - 1) < jj
                            s0 = x0 + 2 if pred_earlier else x0
                            n = x0 + W + 2 - s0
                            nc.vector.tensor_mul(
                                _win2(q12[:], s0, SW, n),
                                _win2(LRh[:], s0, SW, n),
                                _win2(src[:], H0 - 2 + s0, 2, n))
                            nc.vector.tensor_mul(
                                _win2(t34[:], x0, F, W),
                                _win2(CDU[:], x0, F, W),
                                _win2(src[:], x0, 2 * H0, W))
                            a, e = x0, x0 + W
                            mm = nc.tensor.matmul
                            mm(ps[:, a:e], lhsT=idt[:], rhs=cB[:, a:e], start=True, stop=False)
                            mm(ps[:, a:e], lhsT=idt[:], rhs=q12[:, a + 1:e + 1], start=False, stop=False)
                            mm(ps[:, a:e], lhsT=idt[:], rhs=q12[:, SW + a + 1:SW + e + 1], start=False, stop=False)
                            mm(ps[:, a:e], lhsT=idt[:], rhs=t34[:, a:e], start=False, stop=False)
                            mm(ps[:, a:e], lhsT=idt[:], rhs=t34[:, F + a:F + e], start=False, stop=True)
                            nc.scalar.copy(out=dst[:, H0 + a:H0 + e],
                                           in_=ps[:, a:e])
                            if r == 0:
                                nc.gpsimd.dma_start(out=dst[0:63, F + H0:UW],
                                                    in_=dst[1:64, H0:2 * H0])
                                nc.gpsimd.dma_start(out=dst[64:127, F + H0:UW],
                                                    in_=dst[65:128, H0:2 * H0])
                            if r == 7:
                                nc.sync.dma_start(out=dst[1:128, 0:H0],
                                                  in_=dst[0:127, F:F + H0])
                        continue
                    order = [(it + j) % 4 for j in range(4)]
                    if dve_grouped:
                        # seam op: q-slots {0,1} x {Lh,Rh} (bank 0 reads
                        # slot 1; no chunk window covers it when chunk 0
                        # lands in the second group)
                        nc.vector.tensor_mul(
                            _win2(q12[:], 0, SW, 2),
                            _win2(LRh[:], 0, SW, 2),
                            _win2(src[:], H0 - 2, 2, 2))
                    mm = nc.tensor.matmul
                    for g in range(2):
                        ca, cb = order[2 * g], order[2 * g + 1]
                        xa, dlt = ca * CW, (cb - ca) * CW
                        if dve_grouped:
                            # one op, 2 chunks x 2 windows: q1[s]=Lh[s]*u[s-2]
                            # and q2[s]=Rh[s]*u[s] for both chunks of the
                            # group.  The first group's windows start at the
                            # chunk base (self-covering the seam slots the
                            # group's own matmuls read); the second group's
                            # start 2 slots in (its seams come from the first
                            # group / same op), so it never rewrites slots the
                            # first group's matmuls already read (no WAR
                            # serialization).
                            s0 = xa if g == 0 else xa + 2
                            n = CW + 2 if g == 0 else CW
                            nc.vector.tensor_mul(
                                _win4(q12[:], s0, dlt, SW, n),
                                _win4(LRh[:], s0, dlt, SW, n),
                                _win4(src[:], H0 - 2 + s0, dlt, 2, n))
                            # one op: t3=cD*u_d, t4=cU*u_u for both chunks
                            nc.vector.tensor_mul(
                                _win4(t34[:], xa, dlt, F, CW),
                                _win4(CDU[:], xa, dlt, F, CW),
                                _win4(src[:], xa, dlt, 2 * H0, CW))
                        for c in (ca, cb):
                            x0 = c * CW
                            if not dve_grouped:
                                # a chunk self-covers its two lead q-slots
                                # unless its spatial predecessor is processed
                                # earlier in the same sweep (then the
                                # predecessor's 2-slot tail covers them)
                                j = order.index(c)
                                pred_earlier = c > 0 and order.index(c - 1) < j
                                s0 = x0 + 2 if pred_earlier else x0
                                n = x0 + CW + 2 - s0
                                nc.vector.tensor_mul(
                                    _win2(q12[:], s0, SW, n),
                                    _win2(LRh[:], s0, SW, n),
                                    _win2(src[:], H0 - 2 + s0, 2, n))
                                nc.vector.tensor_mul(
                                    _win2(t34[:], x0, F, CW),
                                    _win2(CDU[:], x0, F, CW),
                                    _win2(src[:], x0, 2 * H0, CW))
                            for k in _bank_order(c):
                                a, e = k * W, k * W + W
                                mm(ps[:, a:e], lhsT=idt[:], rhs=cB[:, a:e], start=True, stop=False)
                                mm(ps[:, a:e], lhsT=idt[:], rhs=q12[:, a + 1:e + 1], start=False, stop=False)
                                mm(ps[:, a:e], lhsT=idt[:], rhs=q12[:, SW + a + 1:SW + e + 1], start=False, stop=False)
                                mm(ps[:, a:e], lhsT=idt[:], rhs=t34[:, a:e], start=False, stop=False)
                                mm(ps[:, a:e], lhsT=idt[:], rhs=t34[:, F + a:F + e], start=False, stop=True)
                                if not act_wide:
                                    nc.scalar.copy(out=dst[:, H0 + a:H0 + a + W],
                                                   in_=ps[:, a:a + W])
                                # halo copies fire straight after the ACT of
                                # the bank holding the boundary row, on the
                                # idle gpsimd DMA queue so they never queue
                                # behind other traffic
                                if k == 0:
                                    nc.gpsimd.dma_start(out=dst[0:63, F + H0:UW],
                                                        in_=dst[1:64, H0:2 * H0])
                                    nc.gpsimd.dma_start(out=dst[64:127, F + H0:UW],
                                                        in_=dst[65:128, H0:2 * H0])
                                if k == 7:
                                    # sync queue: lower issue latency than
                                    # gpsimd's, and it is otherwise idle here
                                    nc.sync.dma_start(out=dst[1:128, 0:H0],
                                                      in_=dst[0:127, F:F + H0])
                            if act_wide:
                                # evacuate both banks at once (PSUM read
                                # spans the bank pair)
                                nc.scalar.copy(out=dst[:, H0 + x0:H0 + x0 + CW],
                                               in_=ps[:, x0:x0 + CW])

                res = bufs[iters % 2]
                ost = wp.tile([P, F], f32, tag="ost", bufs=1)
                nc.vector.tensor_copy(out=ost[:], in_=res[:, H0:F + H0])
                nc.sync.dma_start(out=out[:], in_=ost[:])

    _dedup_ldweights(nc)
    nc.compile()
    return nc


def _get_program(iters: int):
    if iters not in _prog_cache:
        _prog_cache[iters] = _build(iters)
    return _prog_cache[iters]


def _make_in_maps(u, b, K):
    u = np.ascontiguousarray(u, dtype=np.float32)
    b = np.ascontiguousarray(b, dtype=np.float32)
    K = np.ascontiguousarray(K, dtype=np.float32)
    ident = np.eye(P, dtype=np.float16)
    in_maps = []
    for c in range(N_CORES):
        sl = slice(2 * c, 2 * c + 2)
        in_maps.append({
            "u_in": u[sl].reshape(P, F),
            "b_in": b[sl].reshape(P, F),
            "k_in": K[sl].reshape(P, F),
            "ident": ident,
        })
    return in_maps


def kernel(max_iter, u, b, K):
    iters = int(max_iter)
    nc = _get_program(iters)
    in_maps = _make_in_maps(u, b, K)
    res = run_bass_kernel_spmd(nc, in_maps, core_ids=list(range(N_CORES)))
    out = np.concatenate(
        [r["out"].reshape(2, W, W) for r in res.results], axis=0
    ).astype(np.float32)
    return out
